# revision 25
# baseline (speedup 1.0000x reference)
"""PositionalGAT layer on 8 Trainium2 NeuronCores (Bass/Tile) — v5.

Sharding: dst-partitioned nodes; each shard's halo src features are gathered
on the HOST (integer indexing only) and shipped pre-transposed per 128-edge
tile in bf16.  The device computes ft/s_src per edge with dense matmuls
against replicated weights.

v5 changes vs v4 (1.01ms baseline):
 - W's 256 output columns are reordered D-MAJOR/H-MINOR (col = d*H + h) on
   the host, so the per-edge message scaling (ft * ex[h]) is ONE vector op
   per tile with a periodic [(0,64),(1,4)] broadcast AP instead of 4
   per-head ops (the per-head scale-copies were 690us of scalar time).
 - s_dst expansion is FUSED into the fe accumulation chain as a 3rd matmul
   (ohT x sdall -> fe_ps[:,256:260]), so logits come out complete in PSUM;
   the sd_ps/sd_sb staging and per-tile vector adds are gone.
 - leaky-relu + exp run ONCE per 16-tile batch ([128,64]) instead of
   sub-batched; exp writes the denominator columns of rhs_sb directly
   (strided dst).
 - one-hot matrices are shipped as fp8e4m3 (half the DMA bytes, 4x faster
   LDWEIGHTS via FWL); matmuls mix fp8 stationary x bf16 moving.
 - evacuation split: scalar does a plain fe_ps->sbuf bf16 copy, vector does
   the dh-scale at 2x bf16 throughput.
 - batch emission is software-pipelined (fe chains of batch b+1 are emitted
   before the scale/agg of batch b) so the PE never drains between batches.
"""

import numpy as np
import ml_dtypes

import concourse.bass as bass
import concourse.mybir as mybir
import concourse.tile as tile
from concourse import bacc
from concourse.bass_utils import run_bass_kernel_spmd

F32 = mybir.dt.float32
BF16 = mybir.dt.bfloat16
FP8 = mybir.dt.float8e4
BF = ml_dtypes.bfloat16
F8 = ml_dtypes.float8_e4m3

N, E, H, D, P = 50000, 800000, 4, 64, 16
IN = 256
C = IN - P               # 240
HD = H * D               # 256
TBW = HD + H             # 260: ft | logits
NCORES = 8
PT = 128
SB = 16                  # tiles per batch
PAD_REL = 255


def _pad128(x):
    return (x + 127) // 128 * 128


# --------------------------------------------------------------------------
# host-side graph preprocessing (integer indexing / layout only)
# --------------------------------------------------------------------------

def pack_edges(src, dst, n_nodes, n_cores):
    """Group each core's edges per 128-dst-node block, pack densely into
    128-edge tiles.  Uniform tile counts across cores (max per block)."""
    m = n_nodes // n_cores
    nb = (m + PT - 1) // PT
    order = np.argsort(dst, kind="stable")
    ds = dst[order].astype(np.int64)
    ss = src[order].astype(np.int64)
    bounds = np.searchsorted(ds, np.arange(0, n_nodes + 1, m))

    per_core_blocks = []
    tpb = np.ones(nb, np.int64)
    for c in range(n_cores):
        e0, e1 = int(bounds[c]), int(bounds[c + 1])
        dl = ds[e0:e1] - c * m
        sl = ss[e0:e1]
        blk = dl // PT
        o2 = np.argsort(blk, kind="stable")
        dl, sl, blk = dl[o2], sl[o2], blk[o2]
        bb = np.searchsorted(blk, np.arange(nb + 1))
        blocks = []
        for b in range(nb):
            blocks.append((sl[bb[b]:bb[b + 1]], dl[bb[b]:bb[b + 1]] % PT))
            tpb[b] = max(tpb[b], (bb[b + 1] - bb[b] + PT - 1) // PT)
        per_core_blocks.append(blocks)

    offs = np.concatenate([[0], np.cumsum(tpb)])
    T = int(offs[-1])
    B = (T + SB - 1) // SB
    Tp = B * SB

    srci_all = np.zeros((n_cores, Tp * PT), np.int64)
    rel2_all = np.full((n_cores, Tp * PT), PAD_REL, np.int64)
    for c in range(n_cores):
        for b in range(nb):
            s_b, r_b = per_core_blocks[c][b]
            base = int(offs[b]) * PT
            srci_all[c, base:base + len(s_b)] = s_b
            rel2_all[c, base:base + len(s_b)] = r_b

    blk_of = np.full(Tp, -1, np.int64)
    for b in range(nb):
        blk_of[offs[b]:offs[b + 1]] = b

    meta = dict(T=Tp, B=B, nb=nb, m=m,
                offs=[int(x) for x in offs],
                blk_of=[int(x) for x in blk_of])
    return meta, srci_all, rel2_all


# --------------------------------------------------------------------------
# device program (uniform across cores)
# --------------------------------------------------------------------------

def build_program(meta):
    m = meta["m"]
    nb = meta["nb"]
    B = meta["B"]
    MP = _pad128(m)
    blk_of = meta["blk_of"]
    offs = meta["offs"]
    EB = SB * PT             # edges per batch

    nc = bacc.Bacc(None, target_bir_lowering=False, debug=False)
    with tile.TileContext(nc) as tc:
        with tc.tile_pool(name="dram", bufs=1, space="DRAM") as dram:
            feT_t = dram.tile([B, IN, EB], BF16, kind="ExternalInput",
                              name="feT", uniquify=False)
            wa8 = dram.tile([IN, 264], BF16, kind="ExternalInput",
                            name="wa8", uniquify=False)
            featT_own = dram.tile([IN, MP], BF16, kind="ExternalInput",
                                  name="featT_own", uniquify=False)
            feat_own = dram.tile([m, IN], F32, kind="ExternalInput",
                                 name="feat_own", uniquify=False)
            oh_t = dram.tile([B, PT, EB], FP8, kind="ExternalInput",
                             name="oht0", uniquify=False)
            ohT_t = dram.tile([B, PT, EB], FP8, kind="ExternalInput",
                              name="oht1", uniquify=False)
            out_t = dram.tile([m, IN], F32, kind="ExternalOutput",
                              name="out", uniquify=False)

            with tc.tile_pool(name="persist", bufs=1) as pp:
                wa_sb0 = pp.tile([PT, 264], BF16)
                wa_sb1 = pp.tile([PT, 264], BF16)
                fo0 = pp.tile([PT, MP], BF16)
                fo1 = pp.tile([PT, MP], BF16)
                sdall = pp.tile([PT, nb, H], BF16)
                # phase-0 inputs go out on the scalar HWDGE ring so they
                # don't serialize behind the first edge-batch DMAs on sync
                nc.scalar.dma_start(out=wa_sb0[:], in_=wa8[0:PT, :])
                nc.scalar.dma_start(out=wa_sb1[:], in_=wa8[PT:IN, :])
                nc.scalar.dma_start(out=fo0[:], in_=featT_own[0:PT, :])
                nc.scalar.dma_start(out=fo1[:], in_=featT_own[PT:IN, :])

                # ------- phase 0: s_dst for own nodes -> sdall -----------
                with tc.tile_pool(name="ps0", bufs=4, space="PSUM") as psp:
                    for b in range(nb):
                        psd = psp.tile([PT, H], F32, tag="psd")
                        nc.tensor.matmul(out=psd[:],
                                         lhsT=fo0[:, b * PT:(b + 1) * PT],
                                         rhs=wa_sb0[:, 260:264],
                                         start=True, stop=False)
                        nc.tensor.matmul(out=psd[:],
                                         lhsT=fo1[:, b * PT:(b + 1) * PT],
                                         rhs=wa_sb1[:, 260:264],
                                         start=False, stop=True)
                        nc.scalar.copy(out=sdall[:, b, :], in_=psd[:])

                # ------- main: edge batches (software-pipelined) ---------
                with tc.tile_pool(name="pb", bufs=4) as poolb, \
                     tc.tile_pool(name="prhs", bufs=3) as prhs, \
                     tc.tile_pool(name="psc", bufs=2) as pscp, \
                     tc.tile_pool(name="plg", bufs=3) as plg, \
                     tc.tile_pool(name="pt4", bufs=4) as pool4, \
                     tc.tile_pool(name="psf", bufs=6, space="PSUM") as psfe, \
                     tc.tile_pool(name="psch", bufs=2, space="PSUM") as pschain:
                    chain_ps = None
                    pend = None          # deferred scale+agg state per batch

                    def emit_front(bat):
                        """fe chains + evacuation copies + logit staging."""
                        fe0 = poolb.tile([PT, EB], BF16, tag="fe0")
                        fe1 = poolb.tile([PT, EB], BF16, tag="fe1")
                        oh = poolb.tile([PT, SB, PT], FP8, tag="oh")
                        ohT = poolb.tile([PT, SB, PT], FP8, tag="ohT")
                        nc.sync.dma_start(out=fe0[:], in_=feT_t[bat, 0:PT, :])
                        nc.sync.dma_start(out=fe1[:], in_=feT_t[bat, PT:IN, :])
                        nc.sync.dma_start(
                            out=oh[:].rearrange("p j s -> p (j s)"),
                            in_=oh_t[bat, :, :])
                        nc.sync.dma_start(
                            out=ohT[:].rearrange("p j e -> p (j e)"),
                            in_=ohT_t[bat, :, :])

                        sc = pscp.tile([PT, SB, HD], BF16, tag="sc")
                        lgs = plg.tile([PT, SB, H], F32, tag="lgs")
                        rhs_sb = prhs.tile([PT, SB, TBW], BF16, tag="rhs")

                        for j in range(SB):
                            t = bat * SB + j
                            blk = blk_of[t]
                            if blk < 0:
                                continue
                            fe_ps = psfe.tile([PT, TBW], F32, tag="fe")
                            nc.tensor.matmul(
                                out=fe_ps[:],
                                lhsT=fe0[:, j * PT:(j + 1) * PT],
                                rhs=wa_sb0[:, 0:TBW],
                                start=True, stop=False)
                            nc.tensor.matmul(
                                out=fe_ps[:],
                                lhsT=fe1[:, j * PT:(j + 1) * PT],
                                rhs=wa_sb1[:, 0:TBW],
                                start=False, stop=False)
                            # s_dst expansion accumulated onto the s columns
                            nc.tensor.matmul(
                                out=fe_ps[:, HD:TBW],
                                lhsT=ohT[:, j, :],
                                rhs=sdall[:, blk, :],
                                start=False, stop=True,
                                skip_group_check=True)
                            # plain evacuation (scalar) — ex-independent
                            nc.scalar.copy(out=sc[:, j, :],
                                           in_=fe_ps[:, 0:HD])
                            # logit staging (vector)
                            nc.vector.tensor_scalar_mul(
                                out=lgs[:, j, :],
                                in0=fe_ps[:, HD:TBW], scalar1=1.0)

                        # batched leaky-relu + exp -> denominator columns
                        tmp = pool4.tile([PT, SB, H], F32, tag="tmp")
                        nc.vector.tensor_scalar_mul(
                            out=tmp[:], in0=lgs[:], scalar1=0.2)
                        nc.vector.tensor_tensor(
                            out=tmp[:], in0=lgs[:], in1=tmp[:],
                            op=mybir.AluOpType.max)
                        nc.scalar.activation(
                            out=rhs_sb[:, :, HD:TBW], in_=tmp[:],
                            func=mybir.ActivationFunctionType.Exp)
                        return dict(bat=bat, oh=oh, sc=sc, rhs_sb=rhs_sb)

                    def emit_back(st):
                        """dh-scale + aggregation matmuls for a batch."""
                        nonlocal chain_ps
                        bat = st["bat"]
                        oh, sc, rhs_sb = st["oh"], st["sc"], st["rhs_sb"]
                        for j in range(SB):
                            t = bat * SB + j
                            blk = blk_of[t]
                            if blk < 0:
                                continue
                            exrep = (rhs_sb[:, j, HD:TBW]
                                     .rearrange("p (o f) -> p o f", o=1)
                                     .to_broadcast([PT, D, H]))
                            nc.vector.tensor_tensor(
                                out=rhs_sb[:, j, 0:HD].rearrange(
                                    "p (d h) -> p d h", h=H),
                                in0=sc[:, j, :].rearrange(
                                    "p (d h) -> p d h", h=H),
                                in1=exrep, op=mybir.AluOpType.mult)

                            first = (t == offs[blk])
                            last = (t == offs[blk + 1] - 1)
                            if first:
                                chain_ps = pschain.tile(
                                    [PT, TBW], F32, tag="chain")
                            nc.tensor.matmul(out=chain_ps[:],
                                             lhsT=oh[:, j, :],
                                             rhs=rhs_sb[:, j, :],
                                             start=first, stop=last)

                            if last:
                                rn = min(PT, m - blk * PT)
                                ftl = pool4.tile([PT, IN], F32, tag="ftl")
                                nc.sync.dma_start(
                                    out=ftl[:rn, :],
                                    in_=feat_own[
                                        blk * PT:blk * PT + rn, :])
                                dn = pool4.tile([PT, H], F32, tag="dn")
                                nc.vector.tensor_scalar_max(
                                    out=dn[:rn, :],
                                    in0=chain_ps[:rn, HD:TBW],
                                    scalar1=1e-30)
                                rc = pool4.tile([PT, H], F32, tag="rc")
                                nc.vector.reciprocal(rc[:rn, :],
                                                     dn[:rn, :])
                                ot = pool4.tile([PT, IN], F32, tag="ot")
                                rcb = rc[:rn, :].to_broadcast([rn, H, D])
                                nc.vector.tensor_tensor(
                                    out=ot[:rn, :].rearrange(
                                        "p (h d) -> p h d", d=D),
                                    in0=chain_ps[:rn, 0:HD].rearrange(
                                        "p (d h) -> p h d", h=H),
                                    in1=rcb, op=mybir.AluOpType.mult)
                                nc.vector.tensor_tensor(
                                    out=ot[:rn, :], in0=ot[:rn, :],
                                    in1=ftl[:rn, :],
                                    op=mybir.AluOpType.add)
                                nc.sync.dma_start(
                                    out=out_t[blk * PT:blk * PT + rn, :],
                                    in_=ot[:rn, :])

                    for bat in range(B):
                        st = emit_front(bat)
                        if pend is not None:
                            emit_back(pend)
                        pend = st
                    emit_back(pend)

    nc.compile()
    return nc


# --------------------------------------------------------------------------
# host wrapper
# --------------------------------------------------------------------------

def prep_inputs(feat, src, dst, W, attn_src, attn_dst, pos_attn_src,
                pos_attn_dst):
    m = N // NCORES
    MP = _pad128(m)

    feat_bf = feat.astype(BF)

    # W columns reordered d-major/h-minor: col d*H+h = W[:, h*D+d]
    wa8 = np.zeros((IN, 264), np.float32)
    wr = W.reshape(C, H, D)
    wa8[:C, :HD] = wr.transpose(0, 2, 1).reshape(C, HD)
    wa8[:C, HD:HD + 4] = np.einsum("chd,hd->ch", wr, attn_src[0])
    wa8[:C, HD + 4:] = np.einsum("chd,hd->ch", wr, attn_dst[0])
    wa8[C:, HD:HD + 4] = pos_attn_src[0].T
    wa8[C:, HD + 4:] = pos_attn_dst[0].T
    wa8 = wa8.astype(BF)

    meta, srci_all, rel2_all = pack_edges(src, dst, N, NCORES)
    B, Tp = meta["B"], meta["T"]
    EB = SB * PT

    in_maps = []
    for c in range(NCORES):
        # per-tile transposed src features: [B, 256, SB*128]
        g = feat_bf[srci_all[c]]                     # [Tp*128, 256]
        g = g.reshape(B, SB, PT, IN).transpose(0, 3, 1, 2)
        feT = np.ascontiguousarray(g.reshape(B, IN, EB))

        r = rel2_all[c].reshape(B, SB, PT)
        sl = np.arange(PT)
        # oh[b, p(edge), j*128+s] = (rel2(tile j, edge p) == s)
        oh_h = np.ascontiguousarray(
            (r.transpose(0, 2, 1)[:, :, :, None] == sl[None, None, None, :])
            .reshape(B, PT, EB).astype(F8))
        # ohT[b, p(slot), j*128+e] = (p == rel2(tile j, edge e))
        ohT_h = np.ascontiguousarray(
            (sl[None, :, None] == r.reshape(B, 1, EB))
            .astype(F8))

        fo = np.zeros((MP, IN), np.float32)
        fo[:m] = feat[c * m:(c + 1) * m]
        featT_own = np.ascontiguousarray(fo.T).astype(BF)

        in_maps.append(dict(
            feT=feT, wa8=wa8,
            featT_own=featT_own,
            feat_own=np.ascontiguousarray(feat[c * m:(c + 1) * m]),
            oht0=oh_h, oht1=ohT_h,
        ))
    return meta, in_maps


_PROG_CACHE = {}


def run(feat, src, dst, W, attn_src, attn_dst, pos_attn_src, pos_attn_dst,
        trace=False):
    meta, in_maps = prep_inputs(
        feat, src, dst, W, attn_src, attn_dst, pos_attn_src, pos_attn_dst)
    key = (meta["T"], tuple(meta["blk_of"]))
    if key not in _PROG_CACHE:
        _PROG_CACHE[key] = build_program(meta)
    nc = _PROG_CACHE[key]
    res = run_bass_kernel_spmd(nc, in_maps, core_ids=list(range(NCORES)),
                               trace=trace)
    out = np.concatenate([res.results[c]["out"] for c in range(NCORES)], 0)
    return out, res


def kernel(feat, src, dst, W, attn_src, attn_dst, pos_attn_src,
           pos_attn_dst):
    out, _ = run(np.asarray(feat, np.float32), np.asarray(src),
                 np.asarray(dst), np.asarray(W, np.float32),
                 np.asarray(attn_src, np.float32),
                 np.asarray(attn_dst, np.float32),
                 np.asarray(pos_attn_src, np.float32),
                 np.asarray(pos_attn_dst, np.float32))
    return out


# revision 26
# speedup vs baseline: 1.0117x; 1.0117x over previous
"""PositionalGAT layer on 8 Trainium2 NeuronCores (Bass/Tile) — v5.

Sharding: dst-partitioned nodes; each shard's halo src features are gathered
on the HOST (integer indexing only) and shipped pre-transposed per 128-edge
tile in bf16.  The device computes ft/s_src per edge with dense matmuls
against replicated weights.

v5 changes vs v4 (1.01ms baseline):
 - W's 256 output columns are reordered D-MAJOR/H-MINOR (col = d*H + h) on
   the host, so the per-edge message scaling (ft * ex[h]) is ONE vector op
   per tile with a periodic [(0,64),(1,4)] broadcast AP instead of 4
   per-head ops (the per-head scale-copies were 690us of scalar time).
 - s_dst expansion is FUSED into the fe accumulation chain as a 3rd matmul
   (ohT x sdall -> fe_ps[:,256:260]), so logits come out complete in PSUM;
   the sd_ps/sd_sb staging and per-tile vector adds are gone.
 - leaky-relu + exp run ONCE per 16-tile batch ([128,64]) instead of
   sub-batched; exp writes the denominator columns of rhs_sb directly
   (strided dst).
 - one-hot matrices are shipped as fp8e4m3 (half the DMA bytes, 4x faster
   LDWEIGHTS via FWL); matmuls mix fp8 stationary x bf16 moving.
 - evacuation split: scalar does a plain fe_ps->sbuf bf16 copy, vector does
   the dh-scale at 2x bf16 throughput.
 - batch emission is software-pipelined (fe chains of batch b+1 are emitted
   before the scale/agg of batch b) so the PE never drains between batches.
"""

import numpy as np
import ml_dtypes

import concourse.bass as bass
import concourse.mybir as mybir
import concourse.tile as tile
from concourse import bacc
from concourse.bass_utils import run_bass_kernel_spmd

F32 = mybir.dt.float32
BF16 = mybir.dt.bfloat16
FP8 = mybir.dt.float8e4
BF = ml_dtypes.bfloat16
F8 = ml_dtypes.float8_e4m3

N, E, H, D, P = 50000, 800000, 4, 64, 16
IN = 256
C = IN - P               # 240
HD = H * D               # 256
TBW = HD + H             # 260: ft | logits
NCORES = 8
PT = 128
SB = 16                  # tiles per batch
PAD_REL = 255


def _pad128(x):
    return (x + 127) // 128 * 128


# --------------------------------------------------------------------------
# host-side graph preprocessing (integer indexing / layout only)
# --------------------------------------------------------------------------

def pack_edges(src, dst, n_nodes, n_cores):
    """Group each core's edges per 128-dst-node block, pack densely into
    128-edge tiles.  Uniform tile counts across cores (max per block)."""
    m = n_nodes // n_cores
    nb = (m + PT - 1) // PT
    order = np.argsort(dst, kind="stable")
    ds = dst[order].astype(np.int64)
    ss = src[order].astype(np.int64)
    bounds = np.searchsorted(ds, np.arange(0, n_nodes + 1, m))

    per_core_blocks = []
    tpb = np.ones(nb, np.int64)
    for c in range(n_cores):
        e0, e1 = int(bounds[c]), int(bounds[c + 1])
        dl = ds[e0:e1] - c * m
        sl = ss[e0:e1]
        blk = dl // PT
        o2 = np.argsort(blk, kind="stable")
        dl, sl, blk = dl[o2], sl[o2], blk[o2]
        bb = np.searchsorted(blk, np.arange(nb + 1))
        blocks = []
        for b in range(nb):
            blocks.append((sl[bb[b]:bb[b + 1]], dl[bb[b]:bb[b + 1]] % PT))
            tpb[b] = max(tpb[b], (bb[b + 1] - bb[b] + PT - 1) // PT)
        per_core_blocks.append(blocks)

    offs = np.concatenate([[0], np.cumsum(tpb)])
    T = int(offs[-1])
    B = (T + SB - 1) // SB
    Tp = B * SB

    srci_all = np.zeros((n_cores, Tp * PT), np.int64)
    rel2_all = np.full((n_cores, Tp * PT), PAD_REL, np.int64)
    for c in range(n_cores):
        for b in range(nb):
            s_b, r_b = per_core_blocks[c][b]
            base = int(offs[b]) * PT
            srci_all[c, base:base + len(s_b)] = s_b
            rel2_all[c, base:base + len(s_b)] = r_b

    blk_of = np.full(Tp, -1, np.int64)
    for b in range(nb):
        blk_of[offs[b]:offs[b + 1]] = b

    meta = dict(T=Tp, B=B, nb=nb, m=m,
                offs=[int(x) for x in offs],
                blk_of=[int(x) for x in blk_of])
    return meta, srci_all, rel2_all


# --------------------------------------------------------------------------
# device program (uniform across cores)
# --------------------------------------------------------------------------

def build_program(meta):
    m = meta["m"]
    nb = meta["nb"]
    B = meta["B"]
    MP = _pad128(m)
    blk_of = meta["blk_of"]
    offs = meta["offs"]
    EB = SB * PT             # edges per batch

    nc = bacc.Bacc(None, target_bir_lowering=False, debug=False)
    with tile.TileContext(nc) as tc:
        with tc.tile_pool(name="dram", bufs=1, space="DRAM") as dram:
            feT_t = dram.tile([B, IN, EB], BF16, kind="ExternalInput",
                              name="feT", uniquify=False)
            wa8 = dram.tile([IN, 264], BF16, kind="ExternalInput",
                            name="wa8", uniquify=False)
            featT_own = dram.tile([IN, MP], BF16, kind="ExternalInput",
                                  name="featT_own", uniquify=False)
            feat_own = dram.tile([m, IN], F32, kind="ExternalInput",
                                 name="feat_own", uniquify=False)
            oh_t = dram.tile([B, PT, EB], FP8, kind="ExternalInput",
                             name="oht0", uniquify=False)
            ohT_t = dram.tile([B, PT, EB], FP8, kind="ExternalInput",
                              name="oht1", uniquify=False)
            out_t = dram.tile([m, IN], F32, kind="ExternalOutput",
                              name="out", uniquify=False)

            with tc.tile_pool(name="persist", bufs=1) as pp:
                wa_sb0 = pp.tile([PT, 264], BF16)
                wa_sb1 = pp.tile([PT, 264], BF16)
                fo0 = pp.tile([PT, MP], BF16)
                fo1 = pp.tile([PT, MP], BF16)
                sdall = pp.tile([PT, nb, H], BF16)
                nc.sync.dma_start(out=wa_sb0[:], in_=wa8[0:PT, :])
                nc.sync.dma_start(out=wa_sb1[:], in_=wa8[PT:IN, :])
                nc.sync.dma_start(out=fo0[:], in_=featT_own[0:PT, :])
                nc.sync.dma_start(out=fo1[:], in_=featT_own[PT:IN, :])

                # ------- phase 0: s_dst for own nodes -> sdall -----------
                with tc.tile_pool(name="ps0", bufs=4, space="PSUM") as psp:
                    for b in range(nb):
                        psd = psp.tile([PT, H], F32, tag="psd")
                        nc.tensor.matmul(out=psd[:],
                                         lhsT=fo0[:, b * PT:(b + 1) * PT],
                                         rhs=wa_sb0[:, 260:264],
                                         start=True, stop=False)
                        nc.tensor.matmul(out=psd[:],
                                         lhsT=fo1[:, b * PT:(b + 1) * PT],
                                         rhs=wa_sb1[:, 260:264],
                                         start=False, stop=True)
                        nc.scalar.copy(out=sdall[:, b, :], in_=psd[:])

                # ------- main: edge batches (software-pipelined) ---------
                with tc.tile_pool(name="pb", bufs=4) as poolb, \
                     tc.tile_pool(name="prhs", bufs=3) as prhs, \
                     tc.tile_pool(name="psc", bufs=2) as pscp, \
                     tc.tile_pool(name="plg", bufs=3) as plg, \
                     tc.tile_pool(name="pt4", bufs=4) as pool4, \
                     tc.tile_pool(name="psf", bufs=6, space="PSUM") as psfe, \
                     tc.tile_pool(name="psch", bufs=2, space="PSUM") as pschain:
                    chain_ps = None
                    pend = None          # deferred scale+agg state per batch

                    def emit_front(bat):
                        """fe chains + evacuation copies + logit staging."""
                        fe0 = poolb.tile([PT, EB], BF16, tag="fe0")
                        fe1 = poolb.tile([PT, EB], BF16, tag="fe1")
                        oh = poolb.tile([PT, SB, PT], FP8, tag="oh")
                        ohT = poolb.tile([PT, SB, PT], FP8, tag="ohT")
                        nc.sync.dma_start(out=fe0[:], in_=feT_t[bat, 0:PT, :])
                        nc.sync.dma_start(out=fe1[:], in_=feT_t[bat, PT:IN, :])
                        nc.sync.dma_start(
                            out=oh[:].rearrange("p j s -> p (j s)"),
                            in_=oh_t[bat, :, :])
                        nc.sync.dma_start(
                            out=ohT[:].rearrange("p j e -> p (j e)"),
                            in_=ohT_t[bat, :, :])

                        sc = pscp.tile([PT, SB, HD], BF16, tag="sc")
                        lgs = plg.tile([PT, SB, H], F32, tag="lgs")
                        rhs_sb = prhs.tile([PT, SB, TBW], BF16, tag="rhs")

                        for j in range(SB):
                            t = bat * SB + j
                            blk = blk_of[t]
                            if blk < 0:
                                continue
                            fe_ps = psfe.tile([PT, TBW], F32, tag="fe")
                            nc.tensor.matmul(
                                out=fe_ps[:],
                                lhsT=fe0[:, j * PT:(j + 1) * PT],
                                rhs=wa_sb0[:, 0:TBW],
                                start=True, stop=False)
                            nc.tensor.matmul(
                                out=fe_ps[:],
                                lhsT=fe1[:, j * PT:(j + 1) * PT],
                                rhs=wa_sb1[:, 0:TBW],
                                start=False, stop=False)
                            # s_dst expansion accumulated onto the s columns
                            nc.tensor.matmul(
                                out=fe_ps[:, HD:TBW],
                                lhsT=ohT[:, j, :],
                                rhs=sdall[:, blk, :],
                                start=False, stop=True,
                                skip_group_check=True)
                            # plain evacuation (scalar) — ex-independent
                            nc.scalar.copy(out=sc[:, j, :],
                                           in_=fe_ps[:, 0:HD])
                            # logit staging (vector)
                            nc.vector.tensor_scalar_mul(
                                out=lgs[:, j, :],
                                in0=fe_ps[:, HD:TBW], scalar1=1.0)

                        # batched leaky-relu + exp -> denominator columns
                        tmp = pool4.tile([PT, SB, H], F32, tag="tmp")
                        nc.vector.tensor_scalar_mul(
                            out=tmp[:], in0=lgs[:], scalar1=0.2)
                        nc.vector.tensor_tensor(
                            out=tmp[:], in0=lgs[:], in1=tmp[:],
                            op=mybir.AluOpType.max)
                        nc.scalar.activation(
                            out=rhs_sb[:, :, HD:TBW], in_=tmp[:],
                            func=mybir.ActivationFunctionType.Exp)
                        return dict(bat=bat, oh=oh, sc=sc, rhs_sb=rhs_sb)

                    def emit_back(st):
                        """dh-scale + aggregation matmuls for a batch."""
                        nonlocal chain_ps
                        bat = st["bat"]
                        oh, sc, rhs_sb = st["oh"], st["sc"], st["rhs_sb"]
                        for j in range(SB):
                            t = bat * SB + j
                            blk = blk_of[t]
                            if blk < 0:
                                continue
                            exrep = (rhs_sb[:, j, HD:TBW]
                                     .rearrange("p (o f) -> p o f", o=1)
                                     .to_broadcast([PT, D, H]))
                            nc.vector.tensor_tensor(
                                out=rhs_sb[:, j, 0:HD].rearrange(
                                    "p (d h) -> p d h", h=H),
                                in0=sc[:, j, :].rearrange(
                                    "p (d h) -> p d h", h=H),
                                in1=exrep, op=mybir.AluOpType.mult)

                            first = (t == offs[blk])
                            last = (t == offs[blk + 1] - 1)
                            if first:
                                chain_ps = pschain.tile(
                                    [PT, TBW], F32, tag="chain")
                            nc.tensor.matmul(out=chain_ps[:],
                                             lhsT=oh[:, j, :],
                                             rhs=rhs_sb[:, j, :],
                                             start=first, stop=last)

                            if last:
                                rn = min(PT, m - blk * PT)
                                ftl = pool4.tile([PT, IN], F32, tag="ftl")
                                nc.sync.dma_start(
                                    out=ftl[:rn, :],
                                    in_=feat_own[
                                        blk * PT:blk * PT + rn, :])
                                dn = pool4.tile([PT, H], F32, tag="dn")
                                nc.vector.tensor_scalar_max(
                                    out=dn[:rn, :],
                                    in0=chain_ps[:rn, HD:TBW],
                                    scalar1=1e-30)
                                rc = pool4.tile([PT, H], F32, tag="rc")
                                nc.vector.reciprocal(rc[:rn, :],
                                                     dn[:rn, :])
                                ot = pool4.tile([PT, IN], F32, tag="ot")
                                rcb = rc[:rn, :].to_broadcast([rn, H, D])
                                nc.vector.tensor_tensor(
                                    out=ot[:rn, :].rearrange(
                                        "p (h d) -> p h d", d=D),
                                    in0=chain_ps[:rn, 0:HD].rearrange(
                                        "p (d h) -> p h d", h=H),
                                    in1=rcb, op=mybir.AluOpType.mult)
                                nc.vector.tensor_tensor(
                                    out=ot[:rn, :], in0=ot[:rn, :],
                                    in1=ftl[:rn, :],
                                    op=mybir.AluOpType.add)
                                nc.sync.dma_start(
                                    out=out_t[blk * PT:blk * PT + rn, :],
                                    in_=ot[:rn, :])

                    for bat in range(B):
                        st = emit_front(bat)
                        if pend is not None:
                            emit_back(pend)
                        pend = st
                    emit_back(pend)

    nc.compile()
    return nc


# --------------------------------------------------------------------------
# host wrapper
# --------------------------------------------------------------------------

def prep_inputs(feat, src, dst, W, attn_src, attn_dst, pos_attn_src,
                pos_attn_dst):
    m = N // NCORES
    MP = _pad128(m)

    feat_bf = feat.astype(BF)

    # W columns reordered d-major/h-minor: col d*H+h = W[:, h*D+d]
    wa8 = np.zeros((IN, 264), np.float32)
    wr = W.reshape(C, H, D)
    wa8[:C, :HD] = wr.transpose(0, 2, 1).reshape(C, HD)
    wa8[:C, HD:HD + 4] = np.einsum("chd,hd->ch", wr, attn_src[0])
    wa8[:C, HD + 4:] = np.einsum("chd,hd->ch", wr, attn_dst[0])
    wa8[C:, HD:HD + 4] = pos_attn_src[0].T
    wa8[C:, HD + 4:] = pos_attn_dst[0].T
    wa8 = wa8.astype(BF)

    meta, srci_all, rel2_all = pack_edges(src, dst, N, NCORES)
    B, Tp = meta["B"], meta["T"]
    EB = SB * PT

    in_maps = []
    for c in range(NCORES):
        # per-tile transposed src features: [B, 256, SB*128]
        g = feat_bf[srci_all[c]]                     # [Tp*128, 256]
        g = g.reshape(B, SB, PT, IN).transpose(0, 3, 1, 2)
        feT = np.ascontiguousarray(g.reshape(B, IN, EB))

        r = rel2_all[c].reshape(B, SB, PT)
        sl = np.arange(PT)
        # oh[b, p(edge), j*128+s] = (rel2(tile j, edge p) == s)
        oh_h = np.ascontiguousarray(
            (r.transpose(0, 2, 1)[:, :, :, None] == sl[None, None, None, :])
            .reshape(B, PT, EB).astype(F8))
        # ohT[b, p(slot), j*128+e] = (p == rel2(tile j, edge e))
        ohT_h = np.ascontiguousarray(
            (sl[None, :, None] == r.reshape(B, 1, EB))
            .astype(F8))

        fo = np.zeros((MP, IN), np.float32)
        fo[:m] = feat[c * m:(c + 1) * m]
        featT_own = np.ascontiguousarray(fo.T).astype(BF)

        in_maps.append(dict(
            feT=feT, wa8=wa8,
            featT_own=featT_own,
            feat_own=np.ascontiguousarray(feat[c * m:(c + 1) * m]),
            oht0=oh_h, oht1=ohT_h,
        ))
    return meta, in_maps


_PROG_CACHE = {}


def run(feat, src, dst, W, attn_src, attn_dst, pos_attn_src, pos_attn_dst,
        trace=False):
    meta, in_maps = prep_inputs(
        feat, src, dst, W, attn_src, attn_dst, pos_attn_src, pos_attn_dst)
    key = (meta["T"], tuple(meta["blk_of"]))
    if key not in _PROG_CACHE:
        _PROG_CACHE[key] = build_program(meta)
    nc = _PROG_CACHE[key]
    res = run_bass_kernel_spmd(nc, in_maps, core_ids=list(range(NCORES)),
                               trace=trace)
    out = np.concatenate([res.results[c]["out"] for c in range(NCORES)], 0)
    return out, res


def kernel(feat, src, dst, W, attn_src, attn_dst, pos_attn_src,
           pos_attn_dst):
    out, _ = run(np.asarray(feat, np.float32), np.asarray(src),
                 np.asarray(dst), np.asarray(W, np.float32),
                 np.asarray(attn_src, np.float32),
                 np.asarray(attn_dst, np.float32),
                 np.asarray(pos_attn_src, np.float32),
                 np.asarray(pos_attn_dst, np.float32))
    return out


# revision 28
# speedup vs baseline: 1.0188x; 1.0070x over previous
"""PositionalGAT layer on 8 Trainium2 NeuronCores (Bass/Tile) — v5.

Sharding: dst-partitioned nodes; each shard's halo src features are gathered
on the HOST (integer indexing only) and shipped pre-transposed per 128-edge
tile in bf16.  The device computes ft/s_src per edge with dense matmuls
against replicated weights.

v5 changes vs v4 (1.01ms baseline):
 - W's 256 output columns are reordered D-MAJOR/H-MINOR (col = d*H + h) on
   the host, so the per-edge message scaling (ft * ex[h]) is ONE vector op
   per tile with a periodic [(0,64),(1,4)] broadcast AP instead of 4
   per-head ops (the per-head scale-copies were 690us of scalar time).
 - s_dst expansion is FUSED into the fe accumulation chain as a 3rd matmul
   (ohT x sdall -> fe_ps[:,256:260]), so logits come out complete in PSUM;
   the sd_ps/sd_sb staging and per-tile vector adds are gone.
 - leaky-relu + exp run ONCE per 16-tile batch ([128,64]) instead of
   sub-batched; exp writes the denominator columns of rhs_sb directly
   (strided dst).
 - one-hot matrices are shipped as fp8e4m3 (half the DMA bytes, 4x faster
   LDWEIGHTS via FWL); matmuls mix fp8 stationary x bf16 moving.
 - evacuation split: scalar does a plain fe_ps->sbuf bf16 copy, vector does
   the dh-scale at 2x bf16 throughput.
 - batch emission is software-pipelined (fe chains of batch b+1 are emitted
   before the scale/agg of batch b) so the PE never drains between batches.
"""

import numpy as np
import ml_dtypes

import concourse.bass as bass
import concourse.mybir as mybir
import concourse.tile as tile
from concourse import bacc
from concourse.bass_utils import run_bass_kernel_spmd

F32 = mybir.dt.float32
BF16 = mybir.dt.bfloat16
FP8 = mybir.dt.float8e4
BF = ml_dtypes.bfloat16
F8 = ml_dtypes.float8_e4m3

N, E, H, D, P = 50000, 800000, 4, 64, 16
IN = 256
C = IN - P               # 240
HD = H * D               # 256
TBW = HD + H             # 260: ft | logits
NCORES = 8
PT = 128
SB = 16                  # tiles per batch
PAD_REL = 255


def _pad128(x):
    return (x + 127) // 128 * 128


# --------------------------------------------------------------------------
# host-side graph preprocessing (integer indexing / layout only)
# --------------------------------------------------------------------------

def pack_edges(src, dst, n_nodes, n_cores):
    """Group each core's edges per 128-dst-node block, pack densely into
    128-edge tiles.  Uniform tile counts across cores (max per block)."""
    m = n_nodes // n_cores
    nb = (m + PT - 1) // PT
    order = np.argsort(dst, kind="stable")
    ds = dst[order].astype(np.int64)
    ss = src[order].astype(np.int64)
    bounds = np.searchsorted(ds, np.arange(0, n_nodes + 1, m))

    per_core_blocks = []
    tpb = np.ones(nb, np.int64)
    for c in range(n_cores):
        e0, e1 = int(bounds[c]), int(bounds[c + 1])
        dl = ds[e0:e1] - c * m
        sl = ss[e0:e1]
        blk = dl // PT
        o2 = np.argsort(blk, kind="stable")
        dl, sl, blk = dl[o2], sl[o2], blk[o2]
        bb = np.searchsorted(blk, np.arange(nb + 1))
        blocks = []
        for b in range(nb):
            blocks.append((sl[bb[b]:bb[b + 1]], dl[bb[b]:bb[b + 1]] % PT))
            tpb[b] = max(tpb[b], (bb[b + 1] - bb[b] + PT - 1) // PT)
        per_core_blocks.append(blocks)

    offs = np.concatenate([[0], np.cumsum(tpb)])
    T = int(offs[-1])
    B = (T + SB - 1) // SB
    Tp = B * SB

    srci_all = np.zeros((n_cores, Tp * PT), np.int64)
    rel2_all = np.full((n_cores, Tp * PT), PAD_REL, np.int64)
    for c in range(n_cores):
        for b in range(nb):
            s_b, r_b = per_core_blocks[c][b]
            base = int(offs[b]) * PT
            srci_all[c, base:base + len(s_b)] = s_b
            rel2_all[c, base:base + len(s_b)] = r_b

    blk_of = np.full(Tp, -1, np.int64)
    for b in range(nb):
        blk_of[offs[b]:offs[b + 1]] = b

    meta = dict(T=Tp, B=B, nb=nb, m=m,
                offs=[int(x) for x in offs],
                blk_of=[int(x) for x in blk_of])
    return meta, srci_all, rel2_all


# --------------------------------------------------------------------------
# device program (uniform across cores)
# --------------------------------------------------------------------------

def build_program(meta):
    m = meta["m"]
    nb = meta["nb"]
    B = meta["B"]
    MP = _pad128(m)
    blk_of = meta["blk_of"]
    offs = meta["offs"]
    EB = SB * PT             # edges per batch

    nc = bacc.Bacc(None, target_bir_lowering=False, debug=False)
    with tile.TileContext(nc) as tc:
        with tc.tile_pool(name="dram", bufs=1, space="DRAM") as dram:
            feT_t = dram.tile([B, IN, EB], BF16, kind="ExternalInput",
                              name="feT", uniquify=False)
            wa8 = dram.tile([IN, 264], BF16, kind="ExternalInput",
                            name="wa8", uniquify=False)
            featT_own = dram.tile([IN, MP], BF16, kind="ExternalInput",
                                  name="featT_own", uniquify=False)
            feat_own = dram.tile([m, IN], F32, kind="ExternalInput",
                                 name="feat_own", uniquify=False)
            oh_t = dram.tile([B, PT, EB], FP8, kind="ExternalInput",
                             name="oht0", uniquify=False)
            ohT_t = dram.tile([B, PT, EB], FP8, kind="ExternalInput",
                              name="oht1", uniquify=False)
            out_t = dram.tile([m, IN], F32, kind="ExternalOutput",
                              name="out", uniquify=False)

            with tc.tile_pool(name="persist", bufs=1) as pp:
                wa_sb0 = pp.tile([PT, 264], BF16)
                wa_sb1 = pp.tile([PT, 264], BF16)
                fo0 = pp.tile([PT, MP], BF16)
                fo1 = pp.tile([PT, MP], BF16)
                sdall = pp.tile([PT, nb, H], BF16)
                nc.sync.dma_start(out=wa_sb0[:], in_=wa8[0:PT, :])
                nc.sync.dma_start(out=wa_sb1[:], in_=wa8[PT:IN, :])
                # chunked featT_own loads: phase-0 starts after the first
                # chunk instead of stalling the PE on the whole 3.2MB DMA
                NCH = 4
                CW = _pad128(MP // NCH)
                for ch in range(NCH):
                    c0, c1 = ch * CW, min((ch + 1) * CW, MP)
                    if c0 >= c1:
                        continue
                    nc.sync.dma_start(out=fo0[:, c0:c1],
                                      in_=featT_own[0:PT, c0:c1])
                    nc.sync.dma_start(out=fo1[:, c0:c1],
                                      in_=featT_own[PT:IN, c0:c1])

                # ------- phase 0: s_dst for own nodes -> sdall -----------
                with tc.tile_pool(name="ps0", bufs=4, space="PSUM") as psp:
                    for b in range(nb):
                        psd = psp.tile([PT, H], F32, tag="psd")
                        nc.tensor.matmul(out=psd[:],
                                         lhsT=fo0[:, b * PT:(b + 1) * PT],
                                         rhs=wa_sb0[:, 260:264],
                                         start=True, stop=False)
                        nc.tensor.matmul(out=psd[:],
                                         lhsT=fo1[:, b * PT:(b + 1) * PT],
                                         rhs=wa_sb1[:, 260:264],
                                         start=False, stop=True)
                        nc.scalar.copy(out=sdall[:, b, :], in_=psd[:])

                # ------- main: edge batches (software-pipelined) ---------
                with tc.tile_pool(name="pb", bufs=4) as poolb, \
                     tc.tile_pool(name="prhs", bufs=3) as prhs, \
                     tc.tile_pool(name="psc", bufs=2) as pscp, \
                     tc.tile_pool(name="plg", bufs=3) as plg, \
                     tc.tile_pool(name="pt4", bufs=4) as pool4, \
                     tc.tile_pool(name="psf", bufs=6, space="PSUM") as psfe, \
                     tc.tile_pool(name="psch", bufs=2, space="PSUM") as pschain:
                    chain_ps = None
                    pend = None          # deferred scale+agg state per batch

                    def emit_front(bat):
                        """fe chains + evacuation copies + logit staging."""
                        fe0 = poolb.tile([PT, EB], BF16, tag="fe0")
                        fe1 = poolb.tile([PT, EB], BF16, tag="fe1")
                        oh = poolb.tile([PT, SB, PT], FP8, tag="oh")
                        ohT = poolb.tile([PT, SB, PT], FP8, tag="ohT")
                        nc.sync.dma_start(out=fe0[:], in_=feT_t[bat, 0:PT, :])
                        nc.sync.dma_start(out=fe1[:], in_=feT_t[bat, PT:IN, :])
                        nc.sync.dma_start(
                            out=oh[:].rearrange("p j s -> p (j s)"),
                            in_=oh_t[bat, :, :])
                        nc.sync.dma_start(
                            out=ohT[:].rearrange("p j e -> p (j e)"),
                            in_=ohT_t[bat, :, :])

                        sc = pscp.tile([PT, SB, HD], BF16, tag="sc")
                        lgs = plg.tile([PT, SB, H], F32, tag="lgs")
                        rhs_sb = prhs.tile([PT, SB, TBW], BF16, tag="rhs")

                        for j in range(SB):
                            t = bat * SB + j
                            blk = blk_of[t]
                            if blk < 0:
                                continue
                            fe_ps = psfe.tile([PT, TBW], F32, tag="fe")
                            nc.tensor.matmul(
                                out=fe_ps[:],
                                lhsT=fe0[:, j * PT:(j + 1) * PT],
                                rhs=wa_sb0[:, 0:TBW],
                                start=True, stop=False)
                            nc.tensor.matmul(
                                out=fe_ps[:],
                                lhsT=fe1[:, j * PT:(j + 1) * PT],
                                rhs=wa_sb1[:, 0:TBW],
                                start=False, stop=False)
                            # s_dst expansion accumulated onto the s columns
                            nc.tensor.matmul(
                                out=fe_ps[:, HD:TBW],
                                lhsT=ohT[:, j, :],
                                rhs=sdall[:, blk, :],
                                start=False, stop=True,
                                skip_group_check=True)
                            # plain evacuation (scalar) — ex-independent
                            nc.scalar.copy(out=sc[:, j, :],
                                           in_=fe_ps[:, 0:HD])
                            # logit staging (vector)
                            nc.vector.tensor_scalar_mul(
                                out=lgs[:, j, :],
                                in0=fe_ps[:, HD:TBW], scalar1=1.0)

                        # batched leaky-relu + exp -> denominator columns
                        tmp = pool4.tile([PT, SB, H], F32, tag="tmp")
                        nc.vector.tensor_scalar_mul(
                            out=tmp[:], in0=lgs[:], scalar1=0.2)
                        nc.vector.tensor_tensor(
                            out=tmp[:], in0=lgs[:], in1=tmp[:],
                            op=mybir.AluOpType.max)
                        nc.scalar.activation(
                            out=rhs_sb[:, :, HD:TBW], in_=tmp[:],
                            func=mybir.ActivationFunctionType.Exp)
                        return dict(bat=bat, oh=oh, sc=sc, rhs_sb=rhs_sb)

                    def emit_back(st):
                        """dh-scale + aggregation matmuls for a batch."""
                        nonlocal chain_ps
                        bat = st["bat"]
                        oh, sc, rhs_sb = st["oh"], st["sc"], st["rhs_sb"]
                        for j in range(SB):
                            t = bat * SB + j
                            blk = blk_of[t]
                            if blk < 0:
                                continue
                            exrep = (rhs_sb[:, j, HD:TBW]
                                     .rearrange("p (o f) -> p o f", o=1)
                                     .to_broadcast([PT, D, H]))
                            nc.vector.tensor_tensor(
                                out=rhs_sb[:, j, 0:HD].rearrange(
                                    "p (d h) -> p d h", h=H),
                                in0=sc[:, j, :].rearrange(
                                    "p (d h) -> p d h", h=H),
                                in1=exrep, op=mybir.AluOpType.mult)

                            first = (t == offs[blk])
                            last = (t == offs[blk + 1] - 1)
                            if first:
                                chain_ps = pschain.tile(
                                    [PT, TBW], F32, tag="chain")
                            nc.tensor.matmul(out=chain_ps[:],
                                             lhsT=oh[:, j, :],
                                             rhs=rhs_sb[:, j, :],
                                             start=first, stop=last)

                            if last:
                                rn = min(PT, m - blk * PT)
                                ftl = pool4.tile([PT, IN], F32, tag="ftl")
                                nc.sync.dma_start(
                                    out=ftl[:rn, :],
                                    in_=feat_own[
                                        blk * PT:blk * PT + rn, :])
                                dn = pool4.tile([PT, H], F32, tag="dn")
                                nc.vector.tensor_scalar_max(
                                    out=dn[:rn, :],
                                    in0=chain_ps[:rn, HD:TBW],
                                    scalar1=1e-30)
                                rc = pool4.tile([PT, H], F32, tag="rc")
                                nc.vector.reciprocal(rc[:rn, :],
                                                     dn[:rn, :])
                                ot = pool4.tile([PT, IN], F32, tag="ot")
                                rcb = rc[:rn, :].to_broadcast([rn, H, D])
                                nc.vector.tensor_tensor(
                                    out=ot[:rn, :].rearrange(
                                        "p (h d) -> p h d", d=D),
                                    in0=chain_ps[:rn, 0:HD].rearrange(
                                        "p (d h) -> p h d", h=H),
                                    in1=rcb, op=mybir.AluOpType.mult)
                                nc.vector.tensor_tensor(
                                    out=ot[:rn, :], in0=ot[:rn, :],
                                    in1=ftl[:rn, :],
                                    op=mybir.AluOpType.add)
                                nc.sync.dma_start(
                                    out=out_t[blk * PT:blk * PT + rn, :],
                                    in_=ot[:rn, :])

                    for bat in range(B):
                        st = emit_front(bat)
                        if pend is not None:
                            emit_back(pend)
                        pend = st
                    emit_back(pend)

    nc.compile()
    return nc


# --------------------------------------------------------------------------
# host wrapper
# --------------------------------------------------------------------------

def prep_inputs(feat, src, dst, W, attn_src, attn_dst, pos_attn_src,
                pos_attn_dst):
    m = N // NCORES
    MP = _pad128(m)

    feat_bf = feat.astype(BF)

    # W columns reordered d-major/h-minor: col d*H+h = W[:, h*D+d]
    wa8 = np.zeros((IN, 264), np.float32)
    wr = W.reshape(C, H, D)
    wa8[:C, :HD] = wr.transpose(0, 2, 1).reshape(C, HD)
    wa8[:C, HD:HD + 4] = np.einsum("chd,hd->ch", wr, attn_src[0])
    wa8[:C, HD + 4:] = np.einsum("chd,hd->ch", wr, attn_dst[0])
    wa8[C:, HD:HD + 4] = pos_attn_src[0].T
    wa8[C:, HD + 4:] = pos_attn_dst[0].T
    wa8 = wa8.astype(BF)

    meta, srci_all, rel2_all = pack_edges(src, dst, N, NCORES)
    B, Tp = meta["B"], meta["T"]
    EB = SB * PT

    in_maps = []
    for c in range(NCORES):
        # per-tile transposed src features: [B, 256, SB*128]
        g = feat_bf[srci_all[c]]                     # [Tp*128, 256]
        g = g.reshape(B, SB, PT, IN).transpose(0, 3, 1, 2)
        feT = np.ascontiguousarray(g.reshape(B, IN, EB))

        r = rel2_all[c].reshape(B, SB, PT)
        sl = np.arange(PT)
        # oh[b, p(edge), j*128+s] = (rel2(tile j, edge p) == s)
        oh_h = np.ascontiguousarray(
            (r.transpose(0, 2, 1)[:, :, :, None] == sl[None, None, None, :])
            .reshape(B, PT, EB).astype(F8))
        # ohT[b, p(slot), j*128+e] = (p == rel2(tile j, edge e))
        ohT_h = np.ascontiguousarray(
            (sl[None, :, None] == r.reshape(B, 1, EB))
            .astype(F8))

        fo = np.zeros((MP, IN), np.float32)
        fo[:m] = feat[c * m:(c + 1) * m]
        featT_own = np.ascontiguousarray(fo.T).astype(BF)

        in_maps.append(dict(
            feT=feT, wa8=wa8,
            featT_own=featT_own,
            feat_own=np.ascontiguousarray(feat[c * m:(c + 1) * m]),
            oht0=oh_h, oht1=ohT_h,
        ))
    return meta, in_maps


_PROG_CACHE = {}


def run(feat, src, dst, W, attn_src, attn_dst, pos_attn_src, pos_attn_dst,
        trace=False):
    meta, in_maps = prep_inputs(
        feat, src, dst, W, attn_src, attn_dst, pos_attn_src, pos_attn_dst)
    key = (meta["T"], tuple(meta["blk_of"]))
    if key not in _PROG_CACHE:
        _PROG_CACHE[key] = build_program(meta)
    nc = _PROG_CACHE[key]
    res = run_bass_kernel_spmd(nc, in_maps, core_ids=list(range(NCORES)),
                               trace=trace)
    out = np.concatenate([res.results[c]["out"] for c in range(NCORES)], 0)
    return out, res


def kernel(feat, src, dst, W, attn_src, attn_dst, pos_attn_src,
           pos_attn_dst):
    out, _ = run(np.asarray(feat, np.float32), np.asarray(src),
                 np.asarray(dst), np.asarray(W, np.float32),
                 np.asarray(attn_src, np.float32),
                 np.asarray(attn_dst, np.float32),
                 np.asarray(pos_attn_src, np.float32),
                 np.asarray(pos_attn_dst, np.float32))
    return out


# revision 34
# speedup vs baseline: 1.0503x; 1.0309x over previous
"""PositionalGAT layer on 8 Trainium2 NeuronCores (Bass/Tile) — v5.

Sharding: dst-partitioned nodes; each shard's halo src features are gathered
on the HOST (integer indexing only) and shipped pre-transposed per 128-edge
tile in bf16.  The device computes ft/s_src per edge with dense matmuls
against replicated weights.

v5 (381us HW vs the 1015us v4 baseline) — changes vs v4:
 - W's 256 output columns are reordered D-MAJOR/H-MINOR (col = d*H + h) on
   the host, so the per-edge message scaling (ft * ex[h]) is ONE vector op
   per tile with a periodic [(0,64),(1,4)] broadcast AP instead of 4
   per-head ops (the per-head scale-copies were 690us of scalar time).
 - s_dst expansion is FUSED into the fe accumulation chain as a 3rd matmul
   (ohT x sdall -> fe_ps[:,256:260]), so logits come out complete in PSUM;
   the sd_ps/sd_sb staging and per-tile vector adds are gone.
 - leaky-relu + exp run ONCE per 16-tile batch ([128,64]) instead of
   sub-batched; exp writes the denominator columns of rhs_sb directly
   (strided dst).
 - one-hot matrices are shipped as fp8e4m3 (half the DMA bytes, 4x faster
   LDWEIGHTS via FWL); matmuls mix fp8 stationary x bf16 moving.
 - evacuation split: scalar does a plain fe_ps->sbuf bf16 copy, vector does
   the dh-scale at 2x bf16 throughput.
 - batch emission is software-pipelined (fe chains of batch b+1 are emitted
   before the scale/agg of batch b) so the PE never drains between batches.
"""

import numpy as np
import ml_dtypes

import concourse.bass as bass
import concourse.mybir as mybir
import concourse.tile as tile
from concourse import bacc
from concourse.bass_utils import run_bass_kernel_spmd

F32 = mybir.dt.float32
BF16 = mybir.dt.bfloat16
FP8 = mybir.dt.float8e4
BF = ml_dtypes.bfloat16
F8 = ml_dtypes.float8_e4m3

N, E, H, D, P = 50000, 800000, 4, 64, 16
IN = 256
C = IN - P               # 240
HD = H * D               # 256
TBW = HD + H             # 260: ft | logits
NCORES = 8
PT = 128
SB = 16                  # tiles per batch
PAD_REL = 255


def _pad128(x):
    return (x + 127) // 128 * 128


# --------------------------------------------------------------------------
# host-side graph preprocessing (integer indexing / layout only)
# --------------------------------------------------------------------------

def pack_edges(src, dst, n_nodes, n_cores):
    """Group each core's edges per 128-dst-node block, pack densely into
    128-edge tiles.  Uniform tile counts across cores (max per block)."""
    m = n_nodes // n_cores
    nb = (m + PT - 1) // PT
    order = np.argsort(dst, kind="stable")
    ds = dst[order].astype(np.int64)
    ss = src[order].astype(np.int64)
    bounds = np.searchsorted(ds, np.arange(0, n_nodes + 1, m))

    per_core_blocks = []
    tpb = np.ones(nb, np.int64)
    for c in range(n_cores):
        e0, e1 = int(bounds[c]), int(bounds[c + 1])
        dl = ds[e0:e1] - c * m
        sl = ss[e0:e1]
        blk = dl // PT
        o2 = np.argsort(blk, kind="stable")
        dl, sl, blk = dl[o2], sl[o2], blk[o2]
        bb = np.searchsorted(blk, np.arange(nb + 1))
        blocks = []
        for b in range(nb):
            blocks.append((sl[bb[b]:bb[b + 1]], dl[bb[b]:bb[b + 1]] % PT))
            tpb[b] = max(tpb[b], (bb[b + 1] - bb[b] + PT - 1) // PT)
        per_core_blocks.append(blocks)

    offs = np.concatenate([[0], np.cumsum(tpb)])
    T = int(offs[-1])
    B = (T + SB - 1) // SB
    Tp = B * SB

    srci_all = np.zeros((n_cores, Tp * PT), np.int64)
    rel2_all = np.full((n_cores, Tp * PT), PAD_REL, np.int64)
    for c in range(n_cores):
        for b in range(nb):
            s_b, r_b = per_core_blocks[c][b]
            base = int(offs[b]) * PT
            srci_all[c, base:base + len(s_b)] = s_b
            rel2_all[c, base:base + len(s_b)] = r_b

    blk_of = np.full(Tp, -1, np.int64)
    for b in range(nb):
        blk_of[offs[b]:offs[b + 1]] = b

    meta = dict(T=Tp, B=B, nb=nb, m=m,
                offs=[int(x) for x in offs],
                blk_of=[int(x) for x in blk_of])
    return meta, srci_all, rel2_all


# --------------------------------------------------------------------------
# device program (uniform across cores)
# --------------------------------------------------------------------------

def build_program(meta):
    m = meta["m"]
    nb = meta["nb"]
    B = meta["B"]
    MP = _pad128(m)
    blk_of = meta["blk_of"]
    offs = meta["offs"]
    EB = SB * PT             # edges per batch

    nc = bacc.Bacc(None, target_bir_lowering=False, debug=False)
    with tile.TileContext(nc) as tc:
        with tc.tile_pool(name="dram", bufs=1, space="DRAM") as dram:
            feT_t = dram.tile([B, IN, EB], BF16, kind="ExternalInput",
                              name="feT", uniquify=False)
            wa8 = dram.tile([IN, 264], BF16, kind="ExternalInput",
                            name="wa8", uniquify=False)
            featT_own = dram.tile([IN, MP], BF16, kind="ExternalInput",
                                  name="featT_own", uniquify=False)
            feat_own = dram.tile([m, IN], F32, kind="ExternalInput",
                                 name="feat_own", uniquify=False)
            oh_t = dram.tile([B, PT, EB], FP8, kind="ExternalInput",
                             name="oht0", uniquify=False)
            ohT_t = dram.tile([B, PT, EB], FP8, kind="ExternalInput",
                              name="oht1", uniquify=False)
            out_t = dram.tile([m, IN], F32, kind="ExternalOutput",
                              name="out", uniquify=False)

            with tc.tile_pool(name="persist", bufs=1) as pp:
                wa_sb0 = pp.tile([PT, 264], BF16)
                wa_sb1 = pp.tile([PT, 264], BF16)
                fo0 = pp.tile([PT, MP], BF16)
                fo1 = pp.tile([PT, MP], BF16)
                sdall = pp.tile([PT, nb, H], BF16)
                nc.sync.dma_start(out=wa_sb0[:], in_=wa8[0:PT, :])
                nc.sync.dma_start(out=wa_sb1[:], in_=wa8[PT:IN, :])
                # chunked featT_own loads: phase-0 starts after the first
                # chunk instead of stalling the PE on the whole 3.2MB DMA
                NCH = 4
                CW = _pad128(MP // NCH)
                for ch in range(NCH):
                    c0, c1 = ch * CW, min((ch + 1) * CW, MP)
                    if c0 >= c1:
                        continue
                    nc.sync.dma_start(out=fo0[:, c0:c1],
                                      in_=featT_own[0:PT, c0:c1])
                    nc.sync.dma_start(out=fo1[:, c0:c1],
                                      in_=featT_own[PT:IN, c0:c1])

                # ------- phase 0: s_dst for own nodes -> sdall -----------
                with tc.tile_pool(name="ps0", bufs=4, space="PSUM") as psp:
                    for b in range(nb):
                        psd = psp.tile([PT, H], F32, tag="psd")
                        nc.tensor.matmul(out=psd[:],
                                         lhsT=fo0[:, b * PT:(b + 1) * PT],
                                         rhs=wa_sb0[:, 260:264],
                                         start=True, stop=False)
                        nc.tensor.matmul(out=psd[:],
                                         lhsT=fo1[:, b * PT:(b + 1) * PT],
                                         rhs=wa_sb1[:, 260:264],
                                         start=False, stop=True)
                        nc.scalar.copy(out=sdall[:, b, :], in_=psd[:])

                # ------- main: edge batches (software-pipelined) ---------
                with tc.tile_pool(name="pb", bufs=4) as poolb, \
                     tc.tile_pool(name="prhs", bufs=3) as prhs, \
                     tc.tile_pool(name="psc", bufs=2) as pscp, \
                     tc.tile_pool(name="pt4", bufs=4) as pool4, \
                     tc.tile_pool(name="psf", bufs=6, space="PSUM") as psfe, \
                     tc.tile_pool(name="psch", bufs=2, space="PSUM") as pschain:
                    chain_ps = None
                    pend = None          # deferred scale+agg state per batch

                    def emit_front(bat):
                        """fe chains + evacuation copies + logit staging."""
                        fe0 = poolb.tile([PT, EB], BF16, tag="fe0")
                        fe1 = poolb.tile([PT, EB], BF16, tag="fe1")
                        oh = poolb.tile([PT, SB, PT], FP8, tag="oh")
                        ohT = poolb.tile([PT, SB, PT], FP8, tag="ohT")
                        nc.sync.dma_start(out=fe0[:], in_=feT_t[bat, 0:PT, :])
                        nc.sync.dma_start(out=fe1[:], in_=feT_t[bat, PT:IN, :])
                        nc.sync.dma_start(
                            out=oh[:].rearrange("p j s -> p (j s)"),
                            in_=oh_t[bat, :, :])
                        nc.sync.dma_start(
                            out=ohT[:].rearrange("p j e -> p (j e)"),
                            in_=ohT_t[bat, :, :])

                        sc = pscp.tile([PT, SB, TBW], BF16, tag="sc")
                        rhs_sb = prhs.tile([PT, SB, TBW], BF16, tag="rhs")

                        for j in range(SB):
                            t = bat * SB + j
                            blk = blk_of[t]
                            if blk < 0:
                                continue
                            fe_ps = psfe.tile([PT, TBW], F32, tag="fe")
                            nc.tensor.matmul(
                                out=fe_ps[:],
                                lhsT=fe0[:, j * PT:(j + 1) * PT],
                                rhs=wa_sb0[:, 0:TBW],
                                start=True, stop=False)
                            nc.tensor.matmul(
                                out=fe_ps[:],
                                lhsT=fe1[:, j * PT:(j + 1) * PT],
                                rhs=wa_sb1[:, 0:TBW],
                                start=False, stop=False)
                            # s_dst expansion accumulated onto the s columns
                            nc.tensor.matmul(
                                out=fe_ps[:, HD:TBW],
                                lhsT=ohT[:, j, :],
                                rhs=sdall[:, blk, :],
                                start=False, stop=True,
                                skip_group_check=True)
                            # plain evacuation incl. logit columns (bf16);
                            # a few tiles per batch go out on vector to
                            # balance the two engines
                            if j % 5 == 4:
                                nc.vector.tensor_scalar_mul(
                                    out=sc[:, j, :], in0=fe_ps[:],
                                    scalar1=1.0)
                            else:
                                nc.scalar.copy(out=sc[:, j, :],
                                               in_=fe_ps[:])

                        # batched leaky-relu + exp -> denominator columns
                        tmp = pool4.tile([PT, SB, H], F32, tag="tmp")
                        nc.vector.tensor_scalar_mul(
                            out=tmp[:], in0=sc[:, :, HD:TBW], scalar1=0.2)
                        nc.vector.tensor_tensor(
                            out=tmp[:], in0=sc[:, :, HD:TBW], in1=tmp[:],
                            op=mybir.AluOpType.max)
                        nc.scalar.activation(
                            out=rhs_sb[:, :, HD:TBW], in_=tmp[:],
                            func=mybir.ActivationFunctionType.Exp)
                        return dict(bat=bat, oh=oh, sc=sc, rhs_sb=rhs_sb)

                    def emit_back(st):
                        """dh-scale + aggregation matmuls for a batch."""
                        nonlocal chain_ps
                        bat = st["bat"]
                        oh, sc, rhs_sb = st["oh"], st["sc"], st["rhs_sb"]
                        for j in range(SB):
                            t = bat * SB + j
                            blk = blk_of[t]
                            if blk < 0:
                                continue
                            exrep = (rhs_sb[:, j, HD:TBW]
                                     .rearrange("p (o f) -> p o f", o=1)
                                     .to_broadcast([PT, D, H]))
                            nc.vector.tensor_tensor(
                                out=rhs_sb[:, j, 0:HD].rearrange(
                                    "p (d h) -> p d h", h=H),
                                in0=sc[:, j, 0:HD].rearrange(
                                    "p (d h) -> p d h", h=H),
                                in1=exrep, op=mybir.AluOpType.mult)

                            first = (t == offs[blk])
                            last = (t == offs[blk + 1] - 1)
                            if first:
                                chain_ps = pschain.tile(
                                    [PT, TBW], F32, tag="chain")
                            nc.tensor.matmul(out=chain_ps[:],
                                             lhsT=oh[:, j, :],
                                             rhs=rhs_sb[:, j, :],
                                             start=first, stop=last)

                            if last:
                                rn = min(PT, m - blk * PT)
                                ftl = pool4.tile([PT, IN], F32, tag="ftl")
                                nc.sync.dma_start(
                                    out=ftl[:rn, :],
                                    in_=feat_own[
                                        blk * PT:blk * PT + rn, :])
                                dn = pool4.tile([PT, H], F32, tag="dn")
                                nc.vector.tensor_scalar_max(
                                    out=dn[:rn, :],
                                    in0=chain_ps[:rn, HD:TBW],
                                    scalar1=1e-30)
                                rc = pool4.tile([PT, H], F32, tag="rc")
                                nc.vector.reciprocal(rc[:rn, :],
                                                     dn[:rn, :])
                                ot = pool4.tile([PT, IN], F32, tag="ot")
                                rcb = rc[:rn, :].to_broadcast([rn, H, D])
                                nc.vector.tensor_tensor(
                                    out=ot[:rn, :].rearrange(
                                        "p (h d) -> p h d", d=D),
                                    in0=chain_ps[:rn, 0:HD].rearrange(
                                        "p (d h) -> p h d", h=H),
                                    in1=rcb, op=mybir.AluOpType.mult)
                                nc.vector.tensor_tensor(
                                    out=ot[:rn, :], in0=ot[:rn, :],
                                    in1=ftl[:rn, :],
                                    op=mybir.AluOpType.add)
                                nc.sync.dma_start(
                                    out=out_t[blk * PT:blk * PT + rn, :],
                                    in_=ot[:rn, :])

                    for bat in range(B):
                        st = emit_front(bat)
                        if pend is not None:
                            emit_back(pend)
                        pend = st
                    emit_back(pend)

    nc.compile()
    return nc


# --------------------------------------------------------------------------
# host wrapper
# --------------------------------------------------------------------------

def prep_inputs(feat, src, dst, W, attn_src, attn_dst, pos_attn_src,
                pos_attn_dst):
    m = N // NCORES
    MP = _pad128(m)

    feat_bf = feat.astype(BF)

    # W columns reordered d-major/h-minor: col d*H+h = W[:, h*D+d]
    wa8 = np.zeros((IN, 264), np.float32)
    wr = W.reshape(C, H, D)
    wa8[:C, :HD] = wr.transpose(0, 2, 1).reshape(C, HD)
    wa8[:C, HD:HD + 4] = np.einsum("chd,hd->ch", wr, attn_src[0])
    wa8[:C, HD + 4:] = np.einsum("chd,hd->ch", wr, attn_dst[0])
    wa8[C:, HD:HD + 4] = pos_attn_src[0].T
    wa8[C:, HD + 4:] = pos_attn_dst[0].T
    wa8 = wa8.astype(BF)

    meta, srci_all, rel2_all = pack_edges(src, dst, N, NCORES)
    B, Tp = meta["B"], meta["T"]
    EB = SB * PT

    in_maps = []
    for c in range(NCORES):
        # per-tile transposed src features: [B, 256, SB*128]
        g = feat_bf[srci_all[c]]                     # [Tp*128, 256]
        g = g.reshape(B, SB, PT, IN).transpose(0, 3, 1, 2)
        feT = np.ascontiguousarray(g.reshape(B, IN, EB))

        r = rel2_all[c].reshape(B, SB, PT)
        sl = np.arange(PT)
        # oh[b, p(edge), j*128+s] = (rel2(tile j, edge p) == s)
        oh_h = np.ascontiguousarray(
            (r.transpose(0, 2, 1)[:, :, :, None] == sl[None, None, None, :])
            .reshape(B, PT, EB).astype(F8))
        # ohT[b, p(slot), j*128+e] = (p == rel2(tile j, edge e))
        ohT_h = np.ascontiguousarray(
            (sl[None, :, None] == r.reshape(B, 1, EB))
            .astype(F8))

        fo = np.zeros((MP, IN), np.float32)
        fo[:m] = feat[c * m:(c + 1) * m]
        featT_own = np.ascontiguousarray(fo.T).astype(BF)

        in_maps.append(dict(
            feT=feT, wa8=wa8,
            featT_own=featT_own,
            feat_own=np.ascontiguousarray(feat[c * m:(c + 1) * m]),
            oht0=oh_h, oht1=ohT_h,
        ))
    return meta, in_maps


_PROG_CACHE = {}


def run(feat, src, dst, W, attn_src, attn_dst, pos_attn_src, pos_attn_dst,
        trace=False):
    meta, in_maps = prep_inputs(
        feat, src, dst, W, attn_src, attn_dst, pos_attn_src, pos_attn_dst)
    key = (meta["T"], tuple(meta["blk_of"]))
    if key not in _PROG_CACHE:
        _PROG_CACHE[key] = build_program(meta)
    nc = _PROG_CACHE[key]
    res = run_bass_kernel_spmd(nc, in_maps, core_ids=list(range(NCORES)),
                               trace=trace)
    out = np.concatenate([res.results[c]["out"] for c in range(NCORES)], 0)
    return out, res


def kernel(feat, src, dst, W, attn_src, attn_dst, pos_attn_src,
           pos_attn_dst):
    out, _ = run(np.asarray(feat, np.float32), np.asarray(src),
                 np.asarray(dst), np.asarray(W, np.float32),
                 np.asarray(attn_src, np.float32),
                 np.asarray(attn_dst, np.float32),
                 np.asarray(pos_attn_src, np.float32),
                 np.asarray(pos_attn_dst, np.float32))
    return out
